# revision 2
# baseline (speedup 1.0000x reference)
"""Trainium2 Bass kernel for the 32-agent ring message-passing network.

Network (per reference):
  h1 = tanh(states @ W1^T + b1)                 per-agent MLP layer 1
  sf = tanh(h1 @ W2^T + b2)                     per-agent MLP layer 2
  comm[i] = sum_j topo[i,j] (Wv[i,j] @ sf[j] + bv[i,j])
  g = global_state @ Wg^T + bg
  acts = tanh(Wact @ (comm + g) + bact)         per-agent head
  out[b, i*10:(i+1)*10] = acts[i, b]

Key algebraic facts exploited:
  * topo has exactly 3 nonzeros per row (ring: j in {i-1,i,i+1}, each 0.2),
    so comm is 3 neighbor matmuls, not a dense 32x32 pair contraction.
  * Wact folds into Wv on the host: A[i,j] = 0.2 * Wact[i] @ Wv[i,j]
    ([10,36] each), so the comm+head stage contracts 36->10 directly and
    the 5 MB Wv tensor never reaches the device.
  * All biases downstream of sf fold into a single constant row c[i]
    carried in the g-term matmul via a ones-row trick.

Sharding: pure data-parallel over batch. Each of the 8 cores gets
B/8 = 1024 batch rows and all 32 agents; weights replicated; no
collectives. The host pre-transposes activations to feature-major
([feat, batch]) so every DMA is wide rows of contiguous HBM and the
tensor engine needs no on-chip transposes anywhere: the final head
matmul uses sf as the *stationary* operand, which makes PSUM come out
batch-major [128b, 320] exactly as the output wants it.
"""

import numpy as np
import ml_dtypes

import concourse.bacc as bacc
import concourse.mybir as mybir
import concourse.tile as tile
from concourse.bass_utils import run_bass_kernel_spmd

N_AGENT = 32
BATCH = 8192
S_DIM = 48
A_DIM = 10
G_DIM = 128
HID = 36
N_CORES = 8
BC = BATCH // N_CORES          # batch rows per core
N_PAIR = N_AGENT // 2          # agent pairs
NT = 2                         # 512-wide column tiles per core (BC/512)
TW = BC // NT                  # 512
NCHUNK = BC // 128             # 128-wide chunks for the head stage

# Matmul input dtypes. float32r streams at full PE rate for N>=256 with
# ~1e-4 relative error; the head stage (small-N matmuls) runs bf16 for FWL.
MM_DT = mybir.dt.float32r
MM_NP = np.float32
HEAD_DT = mybir.dt.bfloat16
HEAD_NP = ml_dtypes.bfloat16

_CACHE = {}


def _build_program():
    nc = bacc.Bacc("TRN2", target_bir_lowering=False, debug=False,
                   num_devices=N_CORES)

    def param(name, shape, dt_, out=False):
        return nc.declare_dram_parameter(name, list(shape), dt_, isOutput=out)

    st_d = param("st", [N_PAIR, 96, BC], MM_DT)          # statesT, pair-stacked
    gst_d = param("gst", [G_DIM, BC], MM_DT)             # global_state^T slice
    w1b_d = param("w1b", [96, N_PAIR * 72], MM_DT)       # blockdiag W1^T pairs
    b1p_d = param("b1p", [72, N_PAIR], mybir.dt.float32)
    w2b_d = param("w2b", [72, N_PAIR * 72], MM_DT)       # blockdiag W2^T pairs
    b2p_d = param("b2p", [72, N_PAIR], mybir.dt.float32)
    wgt_d = param("wgt", [G_DIM, HID], MM_DT)            # Wg^T
    amov_d = param("amov", [72, N_PAIR * 40], HEAD_DT)   # folded A^T blocks
    wc_d = param("wc", [37, 320], HEAD_DT)               # [WactT rows; c row]
    ones_d = param("ones", [1, BC], HEAD_DT)
    out_d = param("out", [BC, N_AGENT * A_DIM], mybir.dt.float32, out=True)

    with tile.TileContext(nc) as tc:
        with (
            tc.tile_pool(name="wts", bufs=1) as wts,
            tc.tile_pool(name="stp", bufs=4) as stp,
            tc.tile_pool(name="sfp", bufs=1) as sfp,
            tc.tile_pool(name="outp", bufs=3) as outp,
            tc.tile_pool(name="ps", bufs=2, space="PSUM") as psp,
            tc.tile_pool(name="ps_head", bufs=2, space="PSUM") as psh,
        ):
            # --- resident weights -------------------------------------------
            w1b = wts.tile([96, N_PAIR * 72], MM_DT, tag="w1b")
            nc.sync.dma_start(w1b[:], w1b_d[:])
            w2b = wts.tile([72, N_PAIR * 72], MM_DT, tag="w2b")
            nc.sync.dma_start(w2b[:], w2b_d[:])
            b1p = wts.tile([72, N_PAIR], mybir.dt.float32, tag="b1p")
            nc.sync.dma_start(b1p[:], b1p_d[:])
            b2p = wts.tile([72, N_PAIR], mybir.dt.float32, tag="b2p")
            nc.sync.dma_start(b2p[:], b2p_d[:])
            wgt = wts.tile([G_DIM, HID], MM_DT, tag="wgt")
            nc.sync.dma_start(wgt[:], wgt_d[:])
            amov = wts.tile([72, N_PAIR * 40], HEAD_DT, tag="amov")
            nc.sync.dma_start(amov[:], amov_d[:])
            wc = wts.tile([37, 320], HEAD_DT, tag="wc")
            nc.sync.dma_start(wc[:], wc_d[:])
            gst = wts.tile([G_DIM, BC], MM_DT, tag="gst")
            nc.sync.dma_start(gst[:], gst_d[:])

            # --- global feature: Gt = [gs @ Wg^T ; ones] as [37, BC] --------
            gt = wts.tile([37, BC], HEAD_DT, tag="gt")
            nc.sync.dma_start(gt[36:37, :], ones_d[:])
            for t in range(NT):
                ps_g = psp.tile([HID, TW], mybir.dt.float32, tag="psg")
                nc.tensor.matmul(ps_g[:], wgt[:], gst[:, t * TW:(t + 1) * TW],
                                 start=True, stop=True)
                nc.vector.tensor_copy(gt[0:36, t * TW:(t + 1) * TW], ps_g[:])

            # --- per-pair two-layer MLP -> sf tiles (persist, head dtype) ---
            sf_tiles = []
            for p in range(N_PAIR):
                ws = slice(p * 72, (p + 1) * 72)
                st = stp.tile([96, BC], MM_DT, tag="st")
                nc.sync.dma_start(st[:], st_d[p])
                sf = sfp.tile([72, BC], HEAD_DT, tag=f"sf{p}")
                for t in range(NT):
                    sl = slice(t * TW, (t + 1) * TW)
                    ps1 = psp.tile([72, TW], mybir.dt.float32, tag="ps1")
                    nc.tensor.matmul(ps1[:], w1b[:, ws], st[:, sl],
                                     start=True, stop=True)
                    h1 = stp.tile([72, TW], MM_DT, tag="h1")
                    nc.scalar.activation(h1[:], ps1[:],
                                         mybir.ActivationFunctionType.Tanh,
                                         bias=b1p[:, p:p + 1])
                    ps2 = psp.tile([72, TW], mybir.dt.float32, tag="ps2")
                    nc.tensor.matmul(ps2[:], w2b[:, ws], h1[:],
                                     start=True, stop=True)
                    nc.scalar.activation(sf[:, sl], ps2[:],
                                         mybir.ActivationFunctionType.Tanh,
                                         bias=b2p[:, p:p + 1])
                sf_tiles.append(sf)

            # --- head: per 128-batch chunk, accumulate all agents in PSUM ---
            for c in range(NCHUNK):
                cs = slice(c * 128, (c + 1) * 128)
                psum = psh.tile([128, N_AGENT * A_DIM], mybir.dt.float32,
                                tag="php")
                # g-term + folded constant row: writes every column.
                nc.tensor.matmul(psum[:], gt[:, cs], wc[:],
                                 start=True, stop=False)
                for p in range(N_PAIR):
                    sf_c = sf_tiles[p][:, cs]
                    a0 = p * 40
                    if p == 0:
                        # agents 0..2 -> cols 0:30 ; agent 31 -> cols 310:320
                        nc.tensor.matmul(psum[:, 0:30], sf_c,
                                         amov[:, a0 + 10:a0 + 40],
                                         start=False, stop=False)
                        nc.tensor.matmul(psum[:, 310:320], sf_c,
                                         amov[:, a0:a0 + 10],
                                         start=False, stop=False)
                    elif p == N_PAIR - 1:
                        # agents 29..31 -> cols 290:320 ; agent 0 -> cols 0:10
                        nc.tensor.matmul(psum[:, 290:320], sf_c,
                                         amov[:, a0:a0 + 30],
                                         start=False, stop=False)
                        nc.tensor.matmul(psum[:, 0:10], sf_c,
                                         amov[:, a0 + 30:a0 + 40],
                                         start=False, stop=True)
                    else:
                        lo = (2 * p - 1) * A_DIM
                        nc.tensor.matmul(psum[:, lo:lo + 40], sf_c,
                                         amov[:, a0:a0 + 40],
                                         start=False, stop=False)
                ot = outp.tile([128, N_AGENT * A_DIM], mybir.dt.float32,
                               tag="ot")
                nc.scalar.activation(ot[:], psum[:],
                                     mybir.ActivationFunctionType.Tanh)
                nc.sync.dma_start(out_d[cs, :], ot[:])

    nc.compile()
    return nc


def _prep_shared(W1, b1, W2, b2, Wv, bv, Wg, bg, Wact, bact, topo_mat):
    """Host-side weight folding (tiny tensors; done once)."""
    w1b = np.zeros([96, N_PAIR * 72], np.float32)
    w2b = np.zeros([72, N_PAIR * 72], np.float32)
    b1p = np.zeros([72, N_PAIR], np.float32)
    b2p = np.zeros([72, N_PAIR], np.float32)
    for p in range(N_PAIR):
        a0, a1 = 2 * p, 2 * p + 1
        c0 = p * 72
        w1b[0:48, c0:c0 + 36] = W1[a0].T
        w1b[48:96, c0 + 36:c0 + 72] = W1[a1].T
        w2b[0:36, c0:c0 + 36] = W2[a0].T
        w2b[36:72, c0 + 36:c0 + 72] = W2[a1].T
        b1p[0:36, p] = b1[a0]
        b1p[36:72, p] = b1[a1]
        b2p[0:36, p] = b2[a0]
        b2p[36:72, p] = b2[a1]

    # A[i,j] = topo[i,j] * Wact[i] @ Wv[i,j]  (only ring neighbors nonzero)
    def A(i, j):
        return topo_mat[i, j] * (Wact[i] @ Wv[i, j])

    amov = np.zeros([72, N_PAIR * 40], np.float32)
    for p in range(N_PAIR):
        j0, j1 = 2 * p, 2 * p + 1
        for k, i in enumerate([(2 * p - 1) % N_AGENT, 2 * p,
                               2 * p + 1, (2 * p + 2) % N_AGENT]):
            cols = slice(p * 40 + 10 * k, p * 40 + 10 * k + 10)
            if topo_mat[i, j0] != 0:
                amov[0:36, cols] = A(i, j0).T
            if topo_mat[i, j1] != 0:
                amov[36:72, cols] = A(i, j1).T

    # c[i] = Wact[i] @ (bg + sum_j topo[i,j] bv[i,j]) + bact[i]
    beff = np.einsum("ij,ijg->ig", topo_mat, bv)
    wc = np.zeros([37, 320], np.float32)
    for i in range(N_AGENT):
        wc[0:36, 10 * i:10 * i + 10] = Wact[i].T
        wc[36, 10 * i:10 * i + 10] = Wact[i] @ (bg + beff[i]) + bact[i]

    return {
        "w1b": w1b.astype(MM_NP), "b1p": b1p, "w2b": w2b.astype(MM_NP),
        "b2p": b2p, "wgt": np.ascontiguousarray(Wg.T).astype(MM_NP),
        "amov": amov.astype(HEAD_NP), "wc": wc.astype(HEAD_NP),
        "ones": np.ones([1, BC], HEAD_NP),
    }


def _make_in_maps(states, global_state, shared):
    stT = np.ascontiguousarray(states.transpose(0, 2, 1))  # [32, 48, 8192]
    gsT = np.ascontiguousarray(global_state.T)             # [128, 8192]
    in_maps = []
    for core in range(N_CORES):
        bs = slice(core * BC, (core + 1) * BC)
        st = stT[:, :, bs].reshape(N_PAIR, 96, BC)
        in_maps.append(dict(shared,
                            st=np.ascontiguousarray(st).astype(MM_NP),
                            gst=np.ascontiguousarray(gsT[:, bs]).astype(MM_NP)))
    return in_maps


def kernel(states, global_state, W1, b1, W2, b2, Wv, bv, Wg, bg,
           Wact, bact, topo_mat):
    states = np.asarray(states, np.float32)
    global_state = np.asarray(global_state, np.float32)
    args = [np.asarray(a, np.float32) for a in
            (W1, b1, W2, b2, Wv, bv, Wg, bg, Wact, bact, topo_mat)]

    if "nc" not in _CACHE:
        _CACHE["nc"] = _build_program()
    nc = _CACHE["nc"]

    in_maps = _make_in_maps(states, global_state, _prep_shared(*args))
    res = run_bass_kernel_spmd(nc, in_maps, list(range(N_CORES)))
    return np.concatenate([r["out"] for r in res.results], axis=0)


# revision 7
# speedup vs baseline: 17649.9444x; 17649.9444x over previous
"""Trainium2 Bass kernel for the 32-agent ring message-passing network.

Network (per reference):
  h1 = tanh(states @ W1^T + b1)                 per-agent MLP layer 1
  sf = tanh(h1 @ W2^T + b2)                     per-agent MLP layer 2
  comm[i] = sum_j topo[i,j] (Wv[i,j] @ sf[j] + bv[i,j])
  g = global_state @ Wg^T + bg
  acts = tanh(Wact @ (comm + g) + bact)         per-agent head
  out[b, i*10:(i+1)*10] = acts[i, b]

Key algebraic facts exploited:
  * topo has exactly 3 nonzeros per row (ring: j in {i-1,i,i+1}, each 0.2),
    so comm is 3 neighbor matmuls, not a dense 32x32 pair contraction.
  * Wact folds into Wv on the host: A[i,j] = 0.2 * Wact[i] @ Wv[i,j]
    ([10,36] each), so the comm+head stage contracts 36->10 directly and
    the 5 MB Wv tensor never reaches the device.
  * All biases downstream of sf fold into a single constant row c[i]
    carried in the g-term matmul via a ones-row trick.

Sharding: pure data-parallel over batch. Each of the 8 cores gets
B/8 = 1024 batch rows and all 32 agents; weights replicated; no
collectives. The host pre-transposes activations to feature-major
([feat, batch]) so every DMA is wide rows of contiguous HBM and the
tensor engine needs no on-chip transposes anywhere: the final head
matmul uses sf as the *stationary* operand, which makes PSUM come out
batch-major [128b, 320] exactly as the output wants it.
"""

import numpy as np
import ml_dtypes

import concourse.bacc as bacc
import concourse.mybir as mybir
import concourse.tile as tile
from concourse.bass_utils import run_bass_kernel_spmd

N_AGENT = 32
BATCH = 8192
S_DIM = 48
A_DIM = 10
G_DIM = 128
HID = 36
N_CORES = 8
BC = BATCH // N_CORES          # batch rows per core
N_PAIR = N_AGENT // 2          # agent pairs
NT = 2                         # 512-wide column tiles per core (BC/512)
TW = BC // NT                  # 512
NCHUNK = BC // 128             # 128-wide chunks for the head stage

# Matmul input dtypes. float32r streams at full PE rate for N>=256 with
# ~1e-4 relative error; the head stage (small-N matmuls) runs bf16 for FWL.
MM_DT = mybir.dt.float32r
MM_NP = np.float32
HEAD_DT = mybir.dt.bfloat16
HEAD_NP = ml_dtypes.bfloat16

_CACHE = {}


def _build_program():
    nc = bacc.Bacc("TRN2", target_bir_lowering=False, debug=False,
                   num_devices=N_CORES)

    def param(name, shape, dt_, out=False):
        return nc.declare_dram_parameter(name, list(shape), dt_, isOutput=out)

    st_d = param("st", [96, N_PAIR * BC], MM_DT)         # statesT, k-major
    gst_d = param("gst", [G_DIM, BC], MM_DT)             # global_state^T slice
    w1b_d = param("w1b", [96, N_PAIR * 72], MM_DT)       # blockdiag W1^T pairs
    b1p_d = param("b1p", [72, N_PAIR], mybir.dt.float32)
    w2b_d = param("w2b", [72, N_PAIR * 72], MM_DT)       # blockdiag W2^T pairs
    b2p_d = param("b2p", [72, N_PAIR], mybir.dt.float32)
    wgt_d = param("wgt", [G_DIM, HID], MM_DT)            # Wg^T
    amov_d = param("amov", [72, N_PAIR * 40], HEAD_DT)   # folded A^T blocks
    wc_d = param("wc", [37, 320], HEAD_DT)               # [WactT rows; c row]
    ones_d = param("ones", [1, BC], HEAD_DT)
    out_d = param("out", [BC, N_AGENT * A_DIM], mybir.dt.float32, out=True)

    with tile.TileContext(nc) as tc:
        with (
            tc.tile_pool(name="wts", bufs=1) as wts,
            tc.tile_pool(name="stp", bufs=1) as stp,
            tc.tile_pool(name="mlp", bufs=2) as mlp,
            tc.tile_pool(name="sfp", bufs=1) as sfp,
            tc.tile_pool(name="outp", bufs=3) as outp,
            tc.tile_pool(name="ps", bufs=4, space="PSUM") as psp,
            tc.tile_pool(name="ps_head", bufs=2, space="PSUM") as psh,
        ):
            # --- resident weights -------------------------------------------
            w1b = wts.tile([96, N_PAIR * 72], MM_DT, tag="w1b")
            nc.sync.dma_start(w1b[:], w1b_d[:])
            w2b = wts.tile([72, N_PAIR * 72], MM_DT, tag="w2b")
            nc.sync.dma_start(w2b[:], w2b_d[:])
            b1p = wts.tile([72, N_PAIR], mybir.dt.float32, tag="b1p")
            nc.sync.dma_start(b1p[:], b1p_d[:])
            b2p = wts.tile([72, N_PAIR], mybir.dt.float32, tag="b2p")
            nc.sync.dma_start(b2p[:], b2p_d[:])
            wgt = wts.tile([G_DIM, HID], MM_DT, tag="wgt")
            nc.sync.dma_start(wgt[:], wgt_d[:])
            amov = wts.tile([72, N_PAIR * 40], HEAD_DT, tag="amov")
            nc.sync.dma_start(amov[:], amov_d[:])
            wc = wts.tile([37, 320], HEAD_DT, tag="wc")
            nc.sync.dma_start(wc[:], wc_d[:])
            gst = wts.tile([G_DIM, BC], MM_DT, tag="gst")
            nc.sync.dma_start(gst[:], gst_d[:])

            # --- global feature: Gt = [gs @ Wg^T ; ones] as [37, BC] --------
            gt = wts.tile([37, BC], HEAD_DT, tag="gt")
            nc.sync.dma_start(gt[36:37, :], ones_d[:])
            for t in range(NT):
                ps_g = psp.tile([HID, TW], mybir.dt.float32, tag="psmlp")
                nc.tensor.matmul(ps_g[:], wgt[:], gst[:, t * TW:(t + 1) * TW],
                                 start=True, stop=True)
                nc.vector.tensor_copy(gt[0:36, t * TW:(t + 1) * TW], ps_g[:])

            # statesT resident as one [96, N_PAIR*BC] tile, loaded in 4
            # batched DMAs (1.5 MB each) so L1 starts after the first batch.
            stb = stp.tile([96, N_PAIR * BC], MM_DT, tag="st")
            ST_B = 4
            for a in range(ST_B):
                sl = slice(a * (N_PAIR // ST_B) * BC,
                           (a + 1) * (N_PAIR // ST_B) * BC)
                nc.sync.dma_start(stb[:, sl], st_d[:, sl])

            # --- per-tile: two-layer MLP for all pairs, then head chunks ----
            # Head of tile t only needs sf[:, tile t], so it overlaps with
            # the MLP of tile t+1 on the other engines.
            sf_tiles = [[None] * NT for _ in range(N_PAIR)]
            for t in range(NT):
                sl = slice(t * TW, (t + 1) * TW)
                for p in range(N_PAIR):
                    ws = slice(p * 72, (p + 1) * 72)
                    ps1 = psp.tile([72, TW], mybir.dt.float32, tag="psmlp")
                    nc.tensor.matmul(ps1[:], w1b[:, ws],
                                     stb[:, p * BC + t * TW:p * BC + (t + 1) * TW],
                                     start=True, stop=True)
                    h1 = mlp.tile([72, TW], MM_DT, tag="h1")
                    nc.scalar.activation(h1[:], ps1[:],
                                         mybir.ActivationFunctionType.Tanh,
                                         bias=b1p[:, p:p + 1])
                    ps2 = psp.tile([72, TW], mybir.dt.float32, tag="psmlp")
                    nc.tensor.matmul(ps2[:], w2b[:, ws], h1[:],
                                     start=True, stop=True)
                    sf = sfp.tile([72, TW], HEAD_DT, tag=f"sf{p}_{t}")
                    nc.scalar.activation(sf[:], ps2[:],
                                         mybir.ActivationFunctionType.Tanh,
                                         bias=b2p[:, p:p + 1])
                    sf_tiles[p][t] = sf

                # head: per 128-batch chunk inside this tile
                for cc in range(TW // 128):
                    c0 = t * TW + cc * 128
                    cs = slice(c0, c0 + 128)
                    hs = slice(cc * 128, (cc + 1) * 128)
                    psum = psh.tile([128, N_AGENT * A_DIM], mybir.dt.float32,
                                    tag="php")
                    # g-term + folded constant row: writes every column.
                    nc.tensor.matmul(psum[:], gt[:, cs], wc[:],
                                     start=True, stop=False)
                    for p in range(N_PAIR):
                        sf_c = sf_tiles[p][t][:, hs]
                        a0 = p * 40
                        if p == 0:
                            # agents 0..2 -> cols 0:30 ; agent 31 -> 310:320
                            nc.tensor.matmul(psum[:, 0:30], sf_c,
                                             amov[:, a0 + 10:a0 + 40],
                                             start=False, stop=False)
                            nc.tensor.matmul(psum[:, 310:320], sf_c,
                                             amov[:, a0:a0 + 10],
                                             start=False, stop=False)
                        elif p == N_PAIR - 1:
                            # agents 29..31 -> cols 290:320 ; agent 0 -> 0:10
                            nc.tensor.matmul(psum[:, 290:320], sf_c,
                                             amov[:, a0:a0 + 30],
                                             start=False, stop=False)
                            nc.tensor.matmul(psum[:, 0:10], sf_c,
                                             amov[:, a0 + 30:a0 + 40],
                                             start=False, stop=True)
                        else:
                            lo = (2 * p - 1) * A_DIM
                            nc.tensor.matmul(psum[:, lo:lo + 40], sf_c,
                                             amov[:, a0:a0 + 40],
                                             start=False, stop=False)
                    ot = outp.tile([128, N_AGENT * A_DIM], mybir.dt.float32,
                                   tag="ot")
                    nc.scalar.activation(ot[:], psum[:],
                                         mybir.ActivationFunctionType.Tanh)
                    nc.sync.dma_start(out_d[cs, :], ot[:])

    nc.compile()
    return nc


def _prep_shared(W1, b1, W2, b2, Wv, bv, Wg, bg, Wact, bact, topo_mat):
    """Host-side weight folding (tiny tensors; done once)."""
    w1b = np.zeros([96, N_PAIR * 72], np.float32)
    w2b = np.zeros([72, N_PAIR * 72], np.float32)
    b1p = np.zeros([72, N_PAIR], np.float32)
    b2p = np.zeros([72, N_PAIR], np.float32)
    for p in range(N_PAIR):
        a0, a1 = 2 * p, 2 * p + 1
        c0 = p * 72
        w1b[0:48, c0:c0 + 36] = W1[a0].T
        w1b[48:96, c0 + 36:c0 + 72] = W1[a1].T
        w2b[0:36, c0:c0 + 36] = W2[a0].T
        w2b[36:72, c0 + 36:c0 + 72] = W2[a1].T
        b1p[0:36, p] = b1[a0]
        b1p[36:72, p] = b1[a1]
        b2p[0:36, p] = b2[a0]
        b2p[36:72, p] = b2[a1]

    # A[i,j] = topo[i,j] * Wact[i] @ Wv[i,j]  (only ring neighbors nonzero)
    def A(i, j):
        return topo_mat[i, j] * (Wact[i] @ Wv[i, j])

    amov = np.zeros([72, N_PAIR * 40], np.float32)
    for p in range(N_PAIR):
        j0, j1 = 2 * p, 2 * p + 1
        for k, i in enumerate([(2 * p - 1) % N_AGENT, 2 * p,
                               2 * p + 1, (2 * p + 2) % N_AGENT]):
            cols = slice(p * 40 + 10 * k, p * 40 + 10 * k + 10)
            if topo_mat[i, j0] != 0:
                amov[0:36, cols] = A(i, j0).T
            if topo_mat[i, j1] != 0:
                amov[36:72, cols] = A(i, j1).T

    # c[i] = Wact[i] @ (bg + sum_j topo[i,j] bv[i,j]) + bact[i]
    beff = np.einsum("ij,ijg->ig", topo_mat, bv)
    wc = np.zeros([37, 320], np.float32)
    for i in range(N_AGENT):
        wc[0:36, 10 * i:10 * i + 10] = Wact[i].T
        wc[36, 10 * i:10 * i + 10] = Wact[i] @ (bg + beff[i]) + bact[i]

    return {
        "w1b": w1b.astype(MM_NP), "b1p": b1p, "w2b": w2b.astype(MM_NP),
        "b2p": b2p, "wgt": np.ascontiguousarray(Wg.T).astype(MM_NP),
        "amov": amov.astype(HEAD_NP), "wc": wc.astype(HEAD_NP),
        "ones": np.ones([1, BC], HEAD_NP),
    }


def _make_in_maps(states, global_state, shared):
    stT = np.ascontiguousarray(states.transpose(0, 2, 1))  # [32, 48, 8192]
    gsT = np.ascontiguousarray(global_state.T)             # [128, 8192]
    in_maps = []
    for core in range(N_CORES):
        bs = slice(core * BC, (core + 1) * BC)
        # k-major: st[48u+s, p*BC+b] = states[2p+u, b0+b, s]
        st = (stT[:, :, bs].reshape(N_PAIR, 2, S_DIM, BC)
              .transpose(1, 2, 0, 3).reshape(96, N_PAIR * BC))
        in_maps.append(dict(shared,
                            st=np.ascontiguousarray(st).astype(MM_NP),
                            gst=np.ascontiguousarray(gsT[:, bs]).astype(MM_NP)))
    return in_maps


def kernel(states, global_state, W1, b1, W2, b2, Wv, bv, Wg, bg,
           Wact, bact, topo_mat):
    states = np.asarray(states, np.float32)
    global_state = np.asarray(global_state, np.float32)
    args = [np.asarray(a, np.float32) for a in
            (W1, b1, W2, b2, Wv, bv, Wg, bg, Wact, bact, topo_mat)]

    if "nc" not in _CACHE:
        _CACHE["nc"] = _build_program()
    nc = _CACHE["nc"]

    in_maps = _make_in_maps(states, global_state, _prep_shared(*args))
    res = run_bass_kernel_spmd(nc, in_maps, list(range(N_CORES)))
    return np.concatenate([r["out"] for r in res.results], axis=0)


# revision 11
# speedup vs baseline: 19241.3157x; 1.0902x over previous
"""Trainium2 Bass kernel for the 32-agent ring message-passing network.

Network (per reference):
  h1 = tanh(states @ W1^T + b1)                 per-agent MLP layer 1
  sf = tanh(h1 @ W2^T + b2)                     per-agent MLP layer 2
  comm[i] = sum_j topo[i,j] (Wv[i,j] @ sf[j] + bv[i,j])
  g = global_state @ Wg^T + bg
  acts = tanh(Wact @ (comm + g) + bact)         per-agent head
  out[b, i*10:(i+1)*10] = acts[i, b]

Key algebraic facts exploited:
  * topo has exactly 3 nonzeros per row (ring: j in {i-1,i,i+1}, each 0.2),
    so comm is 3 neighbor matmuls, not a dense 32x32 pair contraction.
  * Wact folds into Wv on the host: A[i,j] = 0.2 * Wact[i] @ Wv[i,j]
    ([10,36] each), so the comm+head stage contracts 36->10 directly and
    the 5 MB Wv tensor never reaches the device.
  * All biases downstream of sf fold into a single constant row c[i]
    carried in the g-term matmul via a ones-row trick.

Sharding: pure data-parallel over batch. Each of the 8 cores gets
B/8 = 1024 batch rows and all 32 agents; weights replicated; no
collectives. The host pre-transposes activations to feature-major
([feat, batch]) so every DMA is wide rows of contiguous HBM and the
tensor engine needs no on-chip transposes anywhere: the final head
matmul uses sf as the *stationary* operand, which makes PSUM come out
batch-major [128b, 320] exactly as the output wants it.
"""

import numpy as np
import ml_dtypes

import concourse.bacc as bacc
import concourse.mybir as mybir
import concourse.tile as tile
from concourse.bass_utils import run_bass_kernel_spmd

N_AGENT = 32
BATCH = 8192
S_DIM = 48
A_DIM = 10
G_DIM = 128
HID = 36
N_CORES = 8
BC = BATCH // N_CORES          # batch rows per core
N_PAIR = N_AGENT // 2          # agent pairs
NT = 2                         # 512-wide column tiles per core (BC/512)
TW = BC // NT                  # 512
NCHUNK = BC // 128             # 128-wide chunks for the head stage

# Matmul input dtypes. float32r streams at full PE rate for N>=256 with
# ~1e-4 relative error; the head stage (small-N matmuls) runs bf16 for FWL.
MM_DT = mybir.dt.float32r
MM_NP = np.float32
HEAD_DT = mybir.dt.bfloat16
HEAD_NP = ml_dtypes.bfloat16

_CACHE = {}


def _build_program():
    nc = bacc.Bacc("TRN2", target_bir_lowering=False, debug=False,
                   num_devices=N_CORES)

    def param(name, shape, dt_, out=False):
        return nc.declare_dram_parameter(name, list(shape), dt_, isOutput=out)

    st_d = param("st", [96, N_PAIR * BC], MM_DT)         # statesT, k-major
    gst_d = param("gst", [G_DIM, BC], MM_DT)             # global_state^T slice
    w1b_d = param("w1b", [96, N_PAIR * 72], MM_DT)       # blockdiag W1^T pairs
    b1p_d = param("b1p", [72, N_PAIR], mybir.dt.float32)
    w2b_d = param("w2b", [72, N_PAIR * 72], MM_DT)       # blockdiag W2^T pairs
    b2p_d = param("b2p", [72, N_PAIR], mybir.dt.float32)
    wgt_d = param("wgt", [G_DIM, HID], MM_DT)            # Wg^T
    amov_d = param("amov", [72, N_PAIR * 40], HEAD_DT)   # folded A^T blocks
    wc_d = param("wc", [37, 320], HEAD_DT)               # [WactT rows; c row]
    ones_d = param("ones", [1, BC], HEAD_DT)
    out_d = param("out", [BC, N_AGENT * A_DIM], mybir.dt.float32, out=True)

    with tile.TileContext(nc) as tc:
        with (
            tc.tile_pool(name="wts", bufs=1) as wts,
            tc.tile_pool(name="stp", bufs=1) as stp,
            tc.tile_pool(name="mlp", bufs=2) as mlp,
            tc.tile_pool(name="sfp", bufs=1) as sfp,
            tc.tile_pool(name="outp", bufs=3) as outp,
            tc.tile_pool(name="ps", bufs=3, space="PSUM") as psp,
            tc.tile_pool(name="ps_head", bufs=2, space="PSUM") as psh,
        ):
            # --- resident weights -------------------------------------------
            w1b = wts.tile([96, N_PAIR * 72], MM_DT, tag="w1b")
            nc.sync.dma_start(w1b[:], w1b_d[:])
            w2b = wts.tile([72, N_PAIR * 72], MM_DT, tag="w2b")
            nc.sync.dma_start(w2b[:], w2b_d[:])
            b1p = wts.tile([72, N_PAIR], mybir.dt.float32, tag="b1p")
            nc.sync.dma_start(b1p[:], b1p_d[:])
            b2p = wts.tile([72, N_PAIR], mybir.dt.float32, tag="b2p")
            nc.sync.dma_start(b2p[:], b2p_d[:])
            wgt = wts.tile([G_DIM, HID], MM_DT, tag="wgt")
            nc.sync.dma_start(wgt[:], wgt_d[:])
            amov = wts.tile([72, N_PAIR * 40], HEAD_DT, tag="amov")
            nc.sync.dma_start(amov[:], amov_d[:])
            wc = wts.tile([37, 320], HEAD_DT, tag="wc")
            nc.sync.dma_start(wc[:], wc_d[:])
            gst = wts.tile([G_DIM, BC], MM_DT, tag="gst")
            nc.sync.dma_start(gst[:], gst_d[:])

            # Dummy tanh so the ACT table set loads during the initial DMAs
            # instead of on the critical path of the first real activation.
            warm = wts.tile([1, 2], mybir.dt.float32, tag="warm")
            nc.gpsimd.memset(warm[:], 0.0)
            nc.scalar.activation(warm[:], warm[:],
                                 mybir.ActivationFunctionType.Tanh)

            # --- global feature: Gt = [gs @ Wg^T ; ones] as [37, BC] --------
            gt = wts.tile([37, BC], HEAD_DT, tag="gt")
            nc.sync.dma_start(gt[36:37, :], ones_d[:])
            for t in range(NT):
                ps_g = psp.tile([HID, TW], mybir.dt.float32, tag="psmlp")
                nc.tensor.matmul(ps_g[:], wgt[:], gst[:, t * TW:(t + 1) * TW],
                                 start=True, stop=True)
                nc.vector.tensor_copy(gt[0:36, t * TW:(t + 1) * TW], ps_g[:])

            # statesT resident as one [96, N_PAIR*BC] tile, loaded in 4
            # batched DMAs (1.5 MB each) so L1 starts after the first batch.
            stb = stp.tile([96, N_PAIR * BC], MM_DT, tag="st")
            ST_B = 4
            for a in range(ST_B):
                sl = slice(a * (N_PAIR // ST_B) * BC,
                           (a + 1) * (N_PAIR // ST_B) * BC)
                nc.sync.dma_start(stb[:, sl], st_d[:, sl])

            # --- two-layer MLP, one FD=1024 activation per (pair, layer) ----
            sf_tiles = []
            for p in range(N_PAIR):
                ws = slice(p * 72, (p + 1) * 72)
                ps1 = psp.tile([72, BC], mybir.dt.float32, tag="psmlp")
                for t in range(NT):
                    nc.tensor.matmul(ps1[:, t * TW:(t + 1) * TW], w1b[:, ws],
                                     stb[:, p * BC + t * TW:p * BC + (t + 1) * TW],
                                     start=True, stop=True)
                h1 = mlp.tile([72, BC], MM_DT, tag="h1")
                nc.scalar.activation(h1[:], ps1[:],
                                     mybir.ActivationFunctionType.Tanh,
                                     bias=b1p[:, p:p + 1])
                ps2 = psp.tile([72, BC], mybir.dt.float32, tag="psmlp")
                for t in range(NT):
                    nc.tensor.matmul(ps2[:, t * TW:(t + 1) * TW], w2b[:, ws],
                                     h1[:, t * TW:(t + 1) * TW],
                                     start=True, stop=True)
                sf = sfp.tile([72, BC], HEAD_DT, tag=f"sf{p}")
                nc.scalar.activation(sf[:], ps2[:],
                                     mybir.ActivationFunctionType.Tanh,
                                     bias=b2p[:, p:p + 1])
                sf_tiles.append(sf)

            # --- head: per 128-batch chunk, accumulate all agents in PSUM ---
            for c in range(NCHUNK):
                cs = slice(c * 128, (c + 1) * 128)
                psum = psh.tile([128, N_AGENT * A_DIM], mybir.dt.float32,
                                tag="php")
                # g-term + folded constant row: writes every column.
                nc.tensor.matmul(psum[:], gt[:, cs], wc[:],
                                 start=True, stop=False)
                for p in range(N_PAIR):
                    sf_c = sf_tiles[p][:, cs]
                    a0 = p * 40
                    if p == 0:
                        # agents 0..2 -> cols 0:30 ; agent 31 -> cols 310:320
                        nc.tensor.matmul(psum[:, 0:30], sf_c,
                                         amov[:, a0 + 10:a0 + 40],
                                         start=False, stop=False)
                        nc.tensor.matmul(psum[:, 310:320], sf_c,
                                         amov[:, a0:a0 + 10],
                                         start=False, stop=False)
                    elif p == N_PAIR - 1:
                        # agents 29..31 -> cols 290:320 ; agent 0 -> cols 0:10
                        nc.tensor.matmul(psum[:, 290:320], sf_c,
                                         amov[:, a0:a0 + 30],
                                         start=False, stop=False)
                        nc.tensor.matmul(psum[:, 0:10], sf_c,
                                         amov[:, a0 + 30:a0 + 40],
                                         start=False, stop=True)
                    else:
                        lo = (2 * p - 1) * A_DIM
                        nc.tensor.matmul(psum[:, lo:lo + 40], sf_c,
                                         amov[:, a0:a0 + 40],
                                         start=False, stop=False)
                ot = outp.tile([128, N_AGENT * A_DIM], mybir.dt.float32,
                               tag="ot")
                nc.scalar.activation(ot[:], psum[:],
                                     mybir.ActivationFunctionType.Tanh)
                nc.sync.dma_start(out_d[cs, :], ot[:])

    nc.compile()
    return nc


def _prep_shared(W1, b1, W2, b2, Wv, bv, Wg, bg, Wact, bact, topo_mat):
    """Host-side weight folding (tiny tensors; done once)."""
    w1b = np.zeros([96, N_PAIR * 72], np.float32)
    w2b = np.zeros([72, N_PAIR * 72], np.float32)
    b1p = np.zeros([72, N_PAIR], np.float32)
    b2p = np.zeros([72, N_PAIR], np.float32)
    for p in range(N_PAIR):
        a0, a1 = 2 * p, 2 * p + 1
        c0 = p * 72
        w1b[0:48, c0:c0 + 36] = W1[a0].T
        w1b[48:96, c0 + 36:c0 + 72] = W1[a1].T
        w2b[0:36, c0:c0 + 36] = W2[a0].T
        w2b[36:72, c0 + 36:c0 + 72] = W2[a1].T
        b1p[0:36, p] = b1[a0]
        b1p[36:72, p] = b1[a1]
        b2p[0:36, p] = b2[a0]
        b2p[36:72, p] = b2[a1]

    # A[i,j] = topo[i,j] * Wact[i] @ Wv[i,j]  (only ring neighbors nonzero)
    def A(i, j):
        return topo_mat[i, j] * (Wact[i] @ Wv[i, j])

    amov = np.zeros([72, N_PAIR * 40], np.float32)
    for p in range(N_PAIR):
        j0, j1 = 2 * p, 2 * p + 1
        for k, i in enumerate([(2 * p - 1) % N_AGENT, 2 * p,
                               2 * p + 1, (2 * p + 2) % N_AGENT]):
            cols = slice(p * 40 + 10 * k, p * 40 + 10 * k + 10)
            if topo_mat[i, j0] != 0:
                amov[0:36, cols] = A(i, j0).T
            if topo_mat[i, j1] != 0:
                amov[36:72, cols] = A(i, j1).T

    # c[i] = Wact[i] @ (bg + sum_j topo[i,j] bv[i,j]) + bact[i]
    beff = np.einsum("ij,ijg->ig", topo_mat, bv)
    wc = np.zeros([37, 320], np.float32)
    for i in range(N_AGENT):
        wc[0:36, 10 * i:10 * i + 10] = Wact[i].T
        wc[36, 10 * i:10 * i + 10] = Wact[i] @ (bg + beff[i]) + bact[i]

    return {
        "w1b": w1b.astype(MM_NP), "b1p": b1p, "w2b": w2b.astype(MM_NP),
        "b2p": b2p, "wgt": np.ascontiguousarray(Wg.T).astype(MM_NP),
        "amov": amov.astype(HEAD_NP), "wc": wc.astype(HEAD_NP),
        "ones": np.ones([1, BC], HEAD_NP),
    }


def _make_in_maps(states, global_state, shared):
    stT = np.ascontiguousarray(states.transpose(0, 2, 1))  # [32, 48, 8192]
    gsT = np.ascontiguousarray(global_state.T)             # [128, 8192]
    in_maps = []
    for core in range(N_CORES):
        bs = slice(core * BC, (core + 1) * BC)
        # k-major: st[48u+s, p*BC+b] = states[2p+u, b0+b, s]
        st = (stT[:, :, bs].reshape(N_PAIR, 2, S_DIM, BC)
              .transpose(1, 2, 0, 3).reshape(96, N_PAIR * BC))
        in_maps.append(dict(shared,
                            st=np.ascontiguousarray(st).astype(MM_NP),
                            gst=np.ascontiguousarray(gsT[:, bs]).astype(MM_NP)))
    return in_maps


def kernel(states, global_state, W1, b1, W2, b2, Wv, bv, Wg, bg,
           Wact, bact, topo_mat):
    states = np.asarray(states, np.float32)
    global_state = np.asarray(global_state, np.float32)
    args = [np.asarray(a, np.float32) for a in
            (W1, b1, W2, b2, Wv, bv, Wg, bg, Wact, bact, topo_mat)]

    if "nc" not in _CACHE:
        _CACHE["nc"] = _build_program()
    nc = _CACHE["nc"]

    in_maps = _make_in_maps(states, global_state, _prep_shared(*args))
    res = run_bass_kernel_spmd(nc, in_maps, list(range(N_CORES)))
    return np.concatenate([r["out"] for r in res.results], axis=0)
